# revision 16
# baseline (speedup 1.0000x reference)
"""Top-2-of-8 MoE (SwiGLU experts) on 8 Trainium2 NeuronCores.

Strategy (expert-parallel sparse dispatch):
  - The dense reference multiplies every unselected expert's output by an
    exact 0.0 routing weight, so only each token's top-2 experts contribute
    to the output. We therefore dispatch each token to exactly its two
    selected experts: 4x less matmul work than the dense formulation.
  - Router (tiny: [8192,1024]x[1024,8]) + top-k + softmax run on host in
    float64; all fp32 backends agree with this selection for these inputs.
  - Core c owns expert c. Host gathers/transposes that expert's tokens into
    xT [DIM, CAP], pre-transposes the expert weights, and the device
    computes yT = (silu(x@w1.T) * (x@w3.T)) @ w2.T * w_routing for its
    tokens. Host scatter-adds the per-expert results in expert order, which
    reproduces the reference's e=0..7 accumulation order exactly.
  - Matmuls run in bf16 (fp32 PSUM accumulation): measured end-to-end
    absmax relative error ~5e-3 vs the fp32 reference.
"""

import numpy as np
import ml_dtypes

B, N, DIM, HID, E, TOPK = 2, 4096, 1024, 2048, 8, 2
NCORES = 8
# Max tokens routed to one expert for the seed-0 inputs is 2175; capacity is
# the next multiple of 128. If an input ever overflows CAP, kernel() falls
# back to multiple launches, so this is a perf choice, not a correctness one.
CAP = 2176

BF16 = ml_dtypes.bfloat16

_NC_CACHE = {}


def _token_chunks(cap):
    """Split [0, cap) into free-dim chunks of <=512 (PE moving-operand max).

    A 384-wide first chunk keeps the startup-critical x load small while its
    stage A still consumes weight columns slower than DMA delivers them."""
    if cap == 2176:
        sizes = [384, 512, 512, 512, 256]
    else:
        sizes, left = [], cap
        while left:
            sizes.append(min(512, left))
            left -= sizes[-1]
    chunks, off = [], 0
    for sz in sizes:
        chunks.append((off, sz))
        off += sz
    return chunks


def _build_moe_core(tc, ap, dim, hid, cap):
    """One core's program: yT[t, :] = silu(x@w1.T) * (x@w3.T) @ w2.T * wt[t].

    Inputs (all pre-transposed on host):
      xt  [dim, cap]  bf16   gathered tokens, transposed
      w1t [dim, hid]  bf16   w1[e].T
      w3t [dim, hid]  bf16   w3[e].T
      w2t [hid, dim]  bf16   w2[e].T
      wt  [128, cap/128] f32 routing weight per token, partition-tiled
    Output:
      y   [cap, dim]  f32
    """
    import concourse.bass as bass
    import concourse.mybir as mybir

    nc = tc.nc
    dt = mybir.dt
    f32 = dt.float32
    bf16 = dt.bfloat16
    AF = mybir.ActivationFunctionType

    nd = dim // 128    # contraction tiles for stage A
    nh = hid // 128    # h tiles (partition dim of hT)
    chunks = _token_chunks(cap)

    xt, w1t, w3t, w2t, wt, y = (ap[k] for k in ("xt", "w1t", "w3t", "w2t", "wt", "y"))

    with (
        tc.tile_pool(name="wpool", bufs=1) as wpool,
        tc.tile_pool(name="xpool", bufs=2) as xpool,
        tc.tile_pool(name="hpool", bufs=2) as hpool,
        tc.tile_pool(name="spool", bufs=4) as spool,
        tc.tile_pool(name="opool", bufs=4) as opool,
        tc.tile_pool(name="psA", bufs=2, space=bass.MemorySpace.PSUM) as psA,
        tc.tile_pool(name="psB", bufs=4, space=bass.MemorySpace.PSUM) as psB,
    ):
        # DMA issue on a sequencer costs ~0.6us per instruction, so the
        # startup-critical loads (chunk-0 x, w1/w3 column 0) are spread
        # round-robin across three otherwise-idle engine queues.
        dmae = [nc.sync, nc.gpsimd, nc.scalar]
        rr = [0]

        def dma0(out, in_):
            dmae[rr[0] % len(dmae)].dma_start(out, in_)
            rr[0] += 1

        wt_t = wpool.tile([128, cap // 128], f32, name="wt_t")
        dma0(wt_t[:], wt[:])

        # PE warm-up: ~3.5us of zero-input matmuls with no DMA dependency.
        # They run during the startup DMA wait and flip the HAM clock gate
        # to 8/8 (2.4GHz) before the real matmul stream begins.
        warm_l = wpool.tile([128, 128], bf16, name="warm_l")
        warm_r = wpool.tile([128, 512], bf16, name="warm_r")
        nc.vector.memset(warm_l[:], 0.0)
        nc.vector.memset(warm_r[:], 0.0)
        warm_p = psB.tile([128, 512], f32, name="py", tag="py")
        for _ in range(8):
            nc.tensor.matmul(warm_p[:], warm_l[:], warm_r[:])

        # Stage-A weight tiles, piecewise along the h axis. The first piece
        # is a tiny [128, 128] so the very first accumulation group's inputs
        # are only ~0.5MB; later pieces are [128, 512].
        pieces = [(0, 128), (128, min(384, hid - 128))]
        pieces += [(s, min(512, hid - s)) for s in range(512, hid, 512)]
        t0_0, tsz_0 = chunks[0]
        xs0 = []
        w1s = {}  # (d, piece_idx) -> tile
        w3s = {}
        # Interleave d-major: x0[d], w1-piece0[d], w3-piece0[d] — the exact
        # dependency order of the first accumulation groups.
        for d in range(nd):
            xd = xpool.tile([128, tsz_0], bf16, name=f"x{d}", tag=f"x{d}")
            dma0(xd[:], xt[d * 128:(d + 1) * 128, t0_0:t0_0 + tsz_0])
            xs0.append(xd)
            s, w = pieces[0]
            t1 = wpool.tile([128, w], bf16, name=f"w1s{d}_0", tag=f"w1s{d}_0")
            dma0(t1[:], w1t[d * 128:(d + 1) * 128, s:s + w])
            w1s[(d, 0)] = t1
            t3 = wpool.tile([128, w], bf16, name=f"w3s{d}_0", tag=f"w3s{d}_0")
            dma0(t3[:], w3t[d * 128:(d + 1) * 128, s:s + w])
            w3s[(d, 0)] = t3
        # Remaining weight pieces, column-major; piece 1 still round-robins
        # across the three queues, the rest go on sync.
        for pi in range(1, len(pieces)):
            s, w = pieces[pi]
            for d in range(nd):
                t1 = wpool.tile([128, w], bf16, name=f"w1s{d}_{pi}",
                                tag=f"w1s{d}_{pi}")
                t3 = wpool.tile([128, w], bf16, name=f"w3s{d}_{pi}",
                                tag=f"w3s{d}_{pi}")
                if pi == 1:
                    dma0(t1[:], w1t[d * 128:(d + 1) * 128, s:s + w])
                    dma0(t3[:], w3t[d * 128:(d + 1) * 128, s:s + w])
                else:
                    nc.sync.dma_start(t1[:], w1t[d * 128:(d + 1) * 128, s:s + w])
                    nc.sync.dma_start(t3[:], w3t[d * 128:(d + 1) * 128, s:s + w])
                w1s[(d, pi)] = t1
                w3s[(d, pi)] = t3

        def wslice(wmap, d, ho):
            col = ho * 128
            for pi, (s, w) in enumerate(pieces):
                if s <= col < s + w:
                    return wmap[(d, pi)][:, col - s:col - s + 128]
            raise AssertionError
        w2s = []
        for h in range(nh):
            t2 = wpool.tile([128, dim], bf16, name=f"w2s{h}", tag=f"w2s{h}")
            nc.sync.dma_start(t2[:], w2t[h * 128:(h + 1) * 128, :])
            w2s.append(t2)

        for ci, (t0, tsz) in enumerate(chunks):
            if ci == 0:
                xs = xs0
            else:
                xs = []
                for d in range(nd):
                    xd = xpool.tile([128, tsz], bf16, name=f"x{d}", tag=f"x{d}")
                    nc.sync.dma_start(xd[:], xt[d * 128:(d + 1) * 128, t0:t0 + tsz])
                    xs.append(xd)

            hts = []
            for ho in range(nh):
                pg = psA.tile([128, tsz], f32, name="pg", tag="pg")
                pu = psA.tile([128, tsz], f32, name="pu", tag="pu")
                for d in range(nd):
                    nc.tensor.matmul(
                        pg[:], wslice(w1s, d, ho), xs[d][:],
                        start=(d == 0), stop=(d == nd - 1),
                    )
                for d in range(nd):
                    nc.tensor.matmul(
                        pu[:], wslice(w3s, d, ho), xs[d][:],
                        start=(d == 0), stop=(d == nd - 1),
                    )
                # silu(g)*u as sigmoid(g)*g*u (CoreSim has no Silu table)
                sg = spool.tile([128, tsz], f32, name="sg", tag="sg")
                nc.scalar.activation(sg[:], pg[:], AF.Sigmoid)
                sh = spool.tile([128, tsz], f32, name="sh", tag="sh")
                nc.vector.tensor_mul(sh[:], sg[:], pg[:])
                ht = hpool.tile([128, tsz], bf16, name=f"h{ho}", tag=f"h{ho}")
                nc.vector.tensor_mul(ht[:], sh[:], pu[:])
                hts.append(ht)

            for tsub in range(tsz // 128):
                gt = (t0 // 128) + tsub  # global token-tile index
                for do in range(0, dim, 512):
                    dsz = min(512, dim - do)
                    py = psB.tile([128, dsz], f32, name="py", tag="py")
                    for ho in range(nh):
                        nc.tensor.matmul(
                            py[:], hts[ho][:, tsub * 128:(tsub + 1) * 128],
                            w2s[ho][:, do:do + dsz],
                            start=(ho == 0), stop=(ho == nh - 1),
                        )
                    yo = opool.tile([128, dsz], f32, name="yo", tag="yo")
                    nc.vector.tensor_scalar_mul(yo[:], py[:], wt_t[:, gt:gt + 1])
                    nc.sync.dma_start(
                        y[gt * 128:(gt + 1) * 128, do:do + dsz], yo[:]
                    )


def build_nc(dim=DIM, hid=HID, cap=CAP, n_cores=NCORES):
    """Build + compile the single-core SPMD bass program (cached)."""
    key = (dim, hid, cap, n_cores)
    if key in _NC_CACHE:
        return _NC_CACHE[key]

    import concourse.tile as tile
    import concourse.mybir as mybir
    from concourse import bacc

    dt = mybir.dt
    nc = bacc.Bacc(
        "TRN2", target_bir_lowering=False, debug=False, num_devices=n_cores
    )
    tensors = {
        "xt": nc.dram_tensor("xt", [dim, cap], dt.bfloat16, kind="ExternalInput"),
        "w1t": nc.dram_tensor("w1t", [dim, hid], dt.bfloat16, kind="ExternalInput"),
        "w3t": nc.dram_tensor("w3t", [dim, hid], dt.bfloat16, kind="ExternalInput"),
        "w2t": nc.dram_tensor("w2t", [hid, dim], dt.bfloat16, kind="ExternalInput"),
        "wt": nc.dram_tensor("wt", [128, cap // 128], dt.float32, kind="ExternalInput"),
        "y": nc.dram_tensor("y", [cap, dim], dt.float32, kind="ExternalOutput"),
    }
    ap = {k: v.ap() for k, v in tensors.items()}
    with tile.TileContext(nc) as tc:
        _build_moe_core(tc, ap, dim, hid, cap)
    nc.compile()
    _NC_CACHE[key] = nc
    return nc


def _route(x_flat, router_w):
    """Host router in float64: logits -> top-2 -> softmax.

    float64 resolves the near-ties the same way as every fp32 backend for
    these inputs (min top2/top3 gap 2.6e-6 checked against jax cpu, jax
    neuron, and numpy fp32)."""
    logits = x_flat.astype(np.float64) @ router_w.astype(np.float64).T
    sel = np.argsort(-logits, axis=1, kind="stable")[:, :TOPK].astype(np.int32)
    g = np.take_along_axis(logits, sel, axis=1)
    ex = np.exp(g - g.max(axis=1, keepdims=True))
    rw = ex / ex.sum(axis=1, keepdims=True)
    return rw.astype(np.float32), sel


def kernel(x, router_w, w1, w3, w2):
    from concourse.bass_utils import run_bass_kernel_spmd

    x = np.ascontiguousarray(np.asarray(x, dtype=np.float32))
    router_w = np.asarray(router_w, dtype=np.float32)
    w1 = np.asarray(w1, dtype=np.float32)
    w3 = np.asarray(w3, dtype=np.float32)
    w2 = np.asarray(w2, dtype=np.float32)

    x_flat = x.reshape(-1, DIM)
    n_tok = x_flat.shape[0]
    rw, sel = _route(x_flat, router_w)

    # Per-expert token lists (ascending token order).
    tok_per_e, wt_per_e = [], []
    for e in range(E):
        tok, kk = np.nonzero(sel == e)
        tok_per_e.append(tok)
        wt_per_e.append(rw[tok, kk])

    nc = build_nc()

    # Pre-transposed bf16 weights, one expert per core.
    w_maps = []
    for e in range(E):
        w_maps.append({
            "w1t": np.ascontiguousarray(w1[e].T).astype(BF16),
            "w3t": np.ascontiguousarray(w3[e].T).astype(BF16),
            "w2t": np.ascontiguousarray(w2[e].T).astype(BF16),
        })

    out = np.zeros((n_tok, DIM), dtype=np.float32)
    n_launches = max(1, -(-max(len(t) for t in tok_per_e) // CAP))
    for launch in range(n_launches):
        in_maps, slices = [], []
        for e in range(E):
            tok = tok_per_e[e][launch * CAP:(launch + 1) * CAP]
            wts = wt_per_e[e][launch * CAP:(launch + 1) * CAP]
            n = len(tok)
            xg = np.zeros((CAP, DIM), dtype=np.float32)
            xg[:n] = x_flat[tok]
            wt_vec = np.zeros(CAP, dtype=np.float32)
            wt_vec[:n] = wts
            in_maps.append({
                "xt": np.ascontiguousarray(xg.T).astype(BF16),
                "wt": np.ascontiguousarray(wt_vec.reshape(CAP // 128, 128).T),
                **w_maps[e],
            })
            slices.append((tok, n))
        res = run_bass_kernel_spmd(nc, in_maps, core_ids=list(range(NCORES)))
        # Expert order == core order, so this add order matches the
        # reference's e=0..7 accumulation.
        for e in range(E):
            tok, n = slices[e]
            if n:
                out[tok] += res.results[e]["y"][:n].astype(np.float32)

    b, n = x.shape[0], x.shape[1]
    return (
        out.reshape(b, n, DIM),
        rw.reshape(b, n, TOPK),
        sel.reshape(b, n, TOPK),
    )


# revision 17
# speedup vs baseline: 1.1982x; 1.1982x over previous
"""Top-2-of-8 MoE (SwiGLU experts) on 8 Trainium2 NeuronCores.

Strategy (expert-parallel sparse dispatch):
  - The dense reference multiplies every unselected expert's output by an
    exact 0.0 routing weight, so only each token's top-2 experts contribute
    to the output. We therefore dispatch each token to exactly its two
    selected experts: 4x less matmul work than the dense formulation.
  - Router (tiny: [8192,1024]x[1024,8]) + top-k + softmax run on host in
    float64; all fp32 backends agree with this selection for these inputs.
  - Core c owns expert c. Host gathers/transposes that expert's tokens into
    xT [DIM, CAP], pre-transposes the expert weights, and the device
    computes yT = (silu(x@w1.T) * (x@w3.T)) @ w2.T * w_routing for its
    tokens. Host scatter-adds the per-expert results in expert order, which
    reproduces the reference's e=0..7 accumulation order exactly.
  - Matmuls run in bf16 (fp32 PSUM accumulation): measured end-to-end
    absmax relative error ~5e-3 vs the fp32 reference.
"""

import numpy as np
import ml_dtypes

B, N, DIM, HID, E, TOPK = 2, 4096, 1024, 2048, 8, 2
NCORES = 8
# Max tokens routed to one expert for the seed-0 inputs is 2175; capacity is
# the next multiple of 128. If an input ever overflows CAP, kernel() falls
# back to multiple launches, so this is a perf choice, not a correctness one.
CAP = 2176

BF16 = ml_dtypes.bfloat16

_NC_CACHE = {}


def _token_chunks(cap):
    """Split [0, cap) into free-dim chunks of <=512 (PE moving-operand max)."""
    chunks = []
    off = 0
    while off < cap:
        sz = min(512, cap - off)
        chunks.append((off, sz))
        off += sz
    return chunks


def _build_moe_core(tc, ap, dim, hid, cap):
    """One core's program: yT[t, :] = silu(x@w1.T) * (x@w3.T) @ w2.T * wt[t].

    Inputs (all pre-transposed on host):
      xt  [dim, cap]  bf16   gathered tokens, transposed
      w1t [dim, hid]  bf16   w1[e].T
      w3t [dim, hid]  bf16   w3[e].T
      w2t [hid, dim]  bf16   w2[e].T
      wt  [128, cap/128] f32 routing weight per token, partition-tiled
    Output:
      y   [cap, dim]  f32
    """
    import concourse.bass as bass
    import concourse.mybir as mybir

    nc = tc.nc
    dt = mybir.dt
    f32 = dt.float32
    bf16 = dt.bfloat16
    AF = mybir.ActivationFunctionType

    nd = dim // 128    # contraction tiles for stage A
    nh = hid // 128    # h tiles (partition dim of hT)
    chunks = _token_chunks(cap)

    xt, w1t, w3t, w2t, wt, y = (ap[k] for k in ("xt", "w1t", "w3t", "w2t", "wt", "y"))

    with (
        tc.tile_pool(name="wpool", bufs=1) as wpool,
        tc.tile_pool(name="xpool", bufs=2) as xpool,
        tc.tile_pool(name="hpool", bufs=2) as hpool,
        tc.tile_pool(name="spool", bufs=4) as spool,
        tc.tile_pool(name="opool", bufs=4) as opool,
        tc.tile_pool(name="psA", bufs=2, space=bass.MemorySpace.PSUM) as psA,
        tc.tile_pool(name="psB", bufs=4, space=bass.MemorySpace.PSUM) as psB,
    ):
        # DMA issue on a sequencer costs ~0.6us per instruction, so the
        # startup-critical loads (chunk-0 x, w1/w3 column 0) are spread
        # round-robin across three otherwise-idle engine queues.
        dmae = [nc.sync, nc.gpsimd, nc.scalar]
        rr = [0]

        def dma0(out, in_):
            dmae[rr[0] % len(dmae)].dma_start(out, in_)
            rr[0] += 1

        wt_t = wpool.tile([128, cap // 128], f32, name="wt_t")
        dma0(wt_t[:], wt[:])

        # Stage-A weight tiles, piecewise along the h axis. The first piece
        # is a tiny [128, 128] so the very first accumulation group's inputs
        # are only ~0.5MB; later pieces are [128, 512].
        pieces = [(0, 128), (128, min(384, hid - 128))]
        pieces += [(s, min(512, hid - s)) for s in range(512, hid, 512)]
        t0_0, tsz_0 = chunks[0]
        xs0 = []
        w1s = {}  # (d, piece_idx) -> tile
        w3s = {}
        # Interleave d-major: x0[d], w1-piece0[d], w3-piece0[d] — the exact
        # dependency order of the first accumulation groups.
        for d in range(nd):
            xd = xpool.tile([128, tsz_0], bf16, name=f"x{d}", tag=f"x{d}")
            dma0(xd[:], xt[d * 128:(d + 1) * 128, t0_0:t0_0 + tsz_0])
            xs0.append(xd)
            s, w = pieces[0]
            t1 = wpool.tile([128, w], bf16, name=f"w1s{d}_0", tag=f"w1s{d}_0")
            dma0(t1[:], w1t[d * 128:(d + 1) * 128, s:s + w])
            w1s[(d, 0)] = t1
            t3 = wpool.tile([128, w], bf16, name=f"w3s{d}_0", tag=f"w3s{d}_0")
            dma0(t3[:], w3t[d * 128:(d + 1) * 128, s:s + w])
            w3s[(d, 0)] = t3
        # Remaining weight pieces, column-major; piece 1 still round-robins
        # across the three queues, the rest go on sync.
        for pi in range(1, len(pieces)):
            s, w = pieces[pi]
            for d in range(nd):
                t1 = wpool.tile([128, w], bf16, name=f"w1s{d}_{pi}",
                                tag=f"w1s{d}_{pi}")
                t3 = wpool.tile([128, w], bf16, name=f"w3s{d}_{pi}",
                                tag=f"w3s{d}_{pi}")
                if pi == 1:
                    dma0(t1[:], w1t[d * 128:(d + 1) * 128, s:s + w])
                    dma0(t3[:], w3t[d * 128:(d + 1) * 128, s:s + w])
                else:
                    nc.sync.dma_start(t1[:], w1t[d * 128:(d + 1) * 128, s:s + w])
                    nc.sync.dma_start(t3[:], w3t[d * 128:(d + 1) * 128, s:s + w])
                w1s[(d, pi)] = t1
                w3s[(d, pi)] = t3

        def wslice(wmap, d, ho):
            col = ho * 128
            for pi, (s, w) in enumerate(pieces):
                if s <= col < s + w:
                    return wmap[(d, pi)][:, col - s:col - s + 128]
            raise AssertionError
        w2s = []
        for h in range(nh):
            t2 = wpool.tile([128, dim], bf16, name=f"w2s{h}", tag=f"w2s{h}")
            nc.sync.dma_start(t2[:], w2t[h * 128:(h + 1) * 128, :])
            w2s.append(t2)

        for ci, (t0, tsz) in enumerate(chunks):
            if ci == 0:
                xs = xs0
            else:
                xs = []
                for d in range(nd):
                    xd = xpool.tile([128, tsz], bf16, name=f"x{d}", tag=f"x{d}")
                    nc.sync.dma_start(xd[:], xt[d * 128:(d + 1) * 128, t0:t0 + tsz])
                    xs.append(xd)

            hts = []
            for ho in range(nh):
                pg = psA.tile([128, tsz], f32, name="pg", tag="pg")
                pu = psA.tile([128, tsz], f32, name="pu", tag="pu")
                for d in range(nd):
                    nc.tensor.matmul(
                        pg[:], wslice(w1s, d, ho), xs[d][:],
                        start=(d == 0), stop=(d == nd - 1),
                    )
                for d in range(nd):
                    nc.tensor.matmul(
                        pu[:], wslice(w3s, d, ho), xs[d][:],
                        start=(d == 0), stop=(d == nd - 1),
                    )
                # silu(g)*u as sigmoid(g)*g*u (CoreSim has no Silu table)
                sg = spool.tile([128, tsz], f32, name="sg", tag="sg")
                nc.scalar.activation(sg[:], pg[:], AF.Sigmoid)
                sh = spool.tile([128, tsz], f32, name="sh", tag="sh")
                nc.vector.tensor_mul(sh[:], sg[:], pg[:])
                ht = hpool.tile([128, tsz], bf16, name=f"h{ho}", tag=f"h{ho}")
                nc.vector.tensor_mul(ht[:], sh[:], pu[:])
                hts.append(ht)

            for tsub in range(tsz // 128):
                gt = (t0 // 128) + tsub  # global token-tile index
                for do in range(0, dim, 512):
                    dsz = min(512, dim - do)
                    py = psB.tile([128, dsz], f32, name="py", tag="py")
                    for ho in range(nh):
                        nc.tensor.matmul(
                            py[:], hts[ho][:, tsub * 128:(tsub + 1) * 128],
                            w2s[ho][:, do:do + dsz],
                            start=(ho == 0), stop=(ho == nh - 1),
                        )
                    yo = opool.tile([128, dsz], f32, name="yo", tag="yo")
                    nc.vector.tensor_scalar_mul(yo[:], py[:], wt_t[:, gt:gt + 1])
                    nc.sync.dma_start(
                        y[gt * 128:(gt + 1) * 128, do:do + dsz], yo[:]
                    )


def build_nc(dim=DIM, hid=HID, cap=CAP, n_cores=NCORES):
    """Build + compile the single-core SPMD bass program (cached)."""
    key = (dim, hid, cap, n_cores)
    if key in _NC_CACHE:
        return _NC_CACHE[key]

    import concourse.tile as tile
    import concourse.mybir as mybir
    from concourse import bacc

    dt = mybir.dt
    nc = bacc.Bacc(
        "TRN2", target_bir_lowering=False, debug=False, num_devices=n_cores
    )
    tensors = {
        "xt": nc.dram_tensor("xt", [dim, cap], dt.bfloat16, kind="ExternalInput"),
        "w1t": nc.dram_tensor("w1t", [dim, hid], dt.bfloat16, kind="ExternalInput"),
        "w3t": nc.dram_tensor("w3t", [dim, hid], dt.bfloat16, kind="ExternalInput"),
        "w2t": nc.dram_tensor("w2t", [hid, dim], dt.bfloat16, kind="ExternalInput"),
        "wt": nc.dram_tensor("wt", [128, cap // 128], dt.float32, kind="ExternalInput"),
        "y": nc.dram_tensor("y", [cap, dim], dt.float32, kind="ExternalOutput"),
    }
    ap = {k: v.ap() for k, v in tensors.items()}
    with tile.TileContext(nc) as tc:
        _build_moe_core(tc, ap, dim, hid, cap)
    nc.compile()
    _NC_CACHE[key] = nc
    return nc


def _route(x_flat, router_w):
    """Host router in float64: logits -> top-2 -> softmax.

    float64 resolves the near-ties the same way as every fp32 backend for
    these inputs (min top2/top3 gap 2.6e-6 checked against jax cpu, jax
    neuron, and numpy fp32)."""
    logits = x_flat.astype(np.float64) @ router_w.astype(np.float64).T
    sel = np.argsort(-logits, axis=1, kind="stable")[:, :TOPK].astype(np.int32)
    g = np.take_along_axis(logits, sel, axis=1)
    ex = np.exp(g - g.max(axis=1, keepdims=True))
    rw = ex / ex.sum(axis=1, keepdims=True)
    return rw.astype(np.float32), sel


def kernel(x, router_w, w1, w3, w2):
    from concourse.bass_utils import run_bass_kernel_spmd

    x = np.ascontiguousarray(np.asarray(x, dtype=np.float32))
    router_w = np.asarray(router_w, dtype=np.float32)
    w1 = np.asarray(w1, dtype=np.float32)
    w3 = np.asarray(w3, dtype=np.float32)
    w2 = np.asarray(w2, dtype=np.float32)

    x_flat = x.reshape(-1, DIM)
    n_tok = x_flat.shape[0]
    rw, sel = _route(x_flat, router_w)

    # Per-expert token lists (ascending token order).
    tok_per_e, wt_per_e = [], []
    for e in range(E):
        tok, kk = np.nonzero(sel == e)
        tok_per_e.append(tok)
        wt_per_e.append(rw[tok, kk])

    nc = build_nc()

    # Pre-transposed bf16 weights, one expert per core.
    w_maps = []
    for e in range(E):
        w_maps.append({
            "w1t": np.ascontiguousarray(w1[e].T).astype(BF16),
            "w3t": np.ascontiguousarray(w3[e].T).astype(BF16),
            "w2t": np.ascontiguousarray(w2[e].T).astype(BF16),
        })

    out = np.zeros((n_tok, DIM), dtype=np.float32)
    n_launches = max(1, -(-max(len(t) for t in tok_per_e) // CAP))
    for launch in range(n_launches):
        in_maps, slices = [], []
        for e in range(E):
            tok = tok_per_e[e][launch * CAP:(launch + 1) * CAP]
            wts = wt_per_e[e][launch * CAP:(launch + 1) * CAP]
            n = len(tok)
            xg = np.zeros((CAP, DIM), dtype=np.float32)
            xg[:n] = x_flat[tok]
            wt_vec = np.zeros(CAP, dtype=np.float32)
            wt_vec[:n] = wts
            in_maps.append({
                "xt": np.ascontiguousarray(xg.T).astype(BF16),
                "wt": np.ascontiguousarray(wt_vec.reshape(CAP // 128, 128).T),
                **w_maps[e],
            })
            slices.append((tok, n))
        res = run_bass_kernel_spmd(nc, in_maps, core_ids=list(range(NCORES)))
        # Expert order == core order, so this add order matches the
        # reference's e=0..7 accumulation.
        for e in range(E):
            tok, n = slices[e]
            if n:
                out[tok] += res.results[e]["y"][:n].astype(np.float32)

    b, n = x.shape[0], x.shape[1]
    return (
        out.reshape(b, n, DIM),
        rw.reshape(b, n, TOPK),
        sel.reshape(b, n, TOPK),
    )
